# revision 5
# baseline (speedup 1.0000x reference)
"""Trainium2 Bass kernel for GQA causal self-attention (B=4, S=2048, D=1024,
16 query heads / 4 kv heads, head_dim 64, QK-RMSNorm + RoPE + per-head q gain).

Sharding: 8 cores = batch(4) x kv-head-pair(2). Each core handles one batch
element and 2 kv heads (= 8 query heads, 512 q dims), producing a partial
output projection against its 512 columns of Wproj; the host adds the two
partials per batch element.

Kernel structure:
- Phase B packs the 4 query heads of each kv group into the matmul free dim
  (q processed in 128-position tiles), so only causally needed k-tiles are
  touched and the causal mask collapses to one 128x128 triangle applied on
  the diagonal tile only (on the otherwise-idle GpSimd engine).
- Both kv groups' scores for a (q-tile, k-tile) pair land in one 2-bank PSUM
  tile and share a single [128,1024] exp; a ones-row appended to V makes the
  PV matmul produce the softmax denominators for free.
- The RMSNorm reciprocal-sqrt runs on DVE (bit-magic + 2 Newton steps) so
  the Activation engine only ever uses the Exp/Square table set (a single
  activation-table load for the whole kernel instead of one per switch).
- A (qkv proj + norm + rope + transpose), B (attention), C (out-proj) are
  interleaved per sequence chunk for cross-engine overlap, with C emitted
  one q-tile behind B so its PSUM dependencies never stall the in-order PE
  queue.
- All host-side inputs are packed into ONE dram blob per core (fewer PJRT
  buffer handles per dispatch measurably cuts per-call overhead on the
  axon-tunneled runtime).
"""

import numpy as np

import concourse.bass as bass
import concourse.mybir as mybir
import concourse.tile as tile
from concourse import bacc
from concourse.bass import ts
from concourse.masks import make_identity

F32 = mybir.dt.float32
F32R = mybir.dt.float32r

S = 2048          # sequence length
DIM = 1024        # model dim (contraction for qkv)
QM = 512          # q dims per core (8 heads x 64)
HD = 64           # head dim
NQH = 8           # local query heads
NKV = 2           # local kv heads
ND = DIM // 128   # 8 d-tiles
NST = S // 128    # 16 s-tiles
NSC = S // 512    # 4 s-chunks
NMT = QM // 128   # 4 q m-tiles
RMS_EPS = 1.1920928955078125e-07

# packed input blob layout (f32 elems): one ExternalInput per core
_BLOB_SIZES = [
    ("xT", DIM * S),
    ("wqt", DIM * QM),
    ("wkvt", DIM * 256),
    ("wpt", QM * DIM),
    ("gain8", 128 * NQH),
    ("cosd", S * 32),
    ("sind", S * 32),
    ("maskd", 128 * 128),
]
BLOB_OFFS = {}
_off = 0
for _n, _sz in _BLOB_SIZES:
    BLOB_OFFS[_n] = (_off, _sz)
    _off += _sz
BLOB_TOTAL = _off

_CACHE = {}


def build_program(reps=1):
    nc = bacc.Bacc("TRN2", target_bir_lowering=False, debug=False)

    blob = nc.dram_tensor("blob", [BLOB_TOTAL], F32, kind="ExternalInput").ap()

    def region(name):
        a, n = BLOB_OFFS[name]
        return blob[a : a + n]

    xT = region("xT").rearrange("(dt p s) -> p dt s", p=128, s=S).bitcast(F32R)
    wqt_b = region("wqt").rearrange("(dt p m) -> p dt m", p=128, m=QM).bitcast(F32R)
    wkvt_b = region("wkvt").rearrange("(dt p m) -> p dt m", p=128, m=256).bitcast(F32R)
    wpt_b = region("wpt").rearrange("(mt p n) -> p mt n", p=128, n=DIM).bitcast(F32R)
    gain8 = region("gain8").rearrange("(p g) -> p g", p=128)
    cosd = region("cosd").rearrange("(st p f) -> p st f", p=128, f=32)
    sind = region("sind").rearrange("(st p f) -> p st f", p=128, f=32)
    maskd = region("maskd").rearrange("(p q) -> p q", p=128).bitcast(F32R)
    out = nc.dram_tensor("out", [S, DIM], F32, kind="ExternalOutput").ap()

    with tile.TileContext(nc) as tc:
        from contextlib import ExitStack

        with ExitStack() as ctx:
            persist = ctx.enter_context(tc.tile_pool(name="persist", bufs=1))
            qt_sb = persist.tile([128, NMT, S], F32R, tag="qt")     # Q^T
            kt_sb = persist.tile([128, S], F32R, tag="kt")          # K^T
            v_sb = persist.tile([128, NST, 130], F32R, tag="v")     # [V|1] x2 kv
            mask_sb = persist.tile([128, 128], F32R, tag="mask")
            cos_sb = persist.tile([128, NST, 32], F32, tag="cos")
            sin_sb = persist.tile([128, NST, 32], F32, tag="sin")
            gain_sb = persist.tile([128, NQH], F32, tag="gain")
            ident = persist.tile([128, 128], F32, tag="ident")
            wqt_sb = persist.tile([128, ND, QM], F32R, tag="wqt")
            wkvt_sb = persist.tile([128, ND, 256], F32R, tag="wkvt")
            wpt_sb = persist.tile([128, NMT, DIM], F32R, tag="wpt")

            def setup():
                # per-iteration input state: weights + tables re-loaded from
                # HBM every iteration so each For_i rep is a complete forward
                # pass (nothing computed is carried across reps). Issued on
                # the gpsimd queue (the idlest) so they overlap the x-chunk
                # loads on the SP queue; wqt's first half leads because it
                # gates the first Q-proj matmuls.
                make_identity(nc, ident[:])
                nc.vector.memset(v_sb[:, :, 64:65].bitcast(mybir.dt.uint32), 0x3F800000)
                nc.vector.memset(v_sb[:, :, 129:130].bitcast(mybir.dt.uint32), 0x3F800000)
                nc.gpsimd.dma_start(wqt_sb[:, 0:2, :], wqt_b[:, 0:2, :])
                nc.gpsimd.dma_start(wkvt_sb[:, 0:2, :], wkvt_b[:, 0:2, :])
                nc.gpsimd.dma_start(wqt_sb[:, 2:ND, :], wqt_b[:, 2:ND, :])
                nc.gpsimd.dma_start(wkvt_sb[:, 2:ND, :], wkvt_b[:, 2:ND, :])
                nc.gpsimd.dma_start(cos_sb[:], cosd)
                nc.gpsimd.dma_start(sin_sb[:], sind)
                nc.gpsimd.dma_start(gain_sb[:], gain8)
                nc.gpsimd.dma_start(mask_sb[:], maskd)
                nc.gpsimd.dma_start(wpt_sb[:], wpt_b)

            # pools
            xpool = ctx.enter_context(tc.tile_pool(name="xpool", bufs=3))
            atmp = ctx.enter_context(tc.tile_pool(name="atmp", bufs=2))
            ppool = ctx.enter_context(tc.tile_pool(name="ppool", bufs=3))
            ytpool = ctx.enter_context(tc.tile_pool(name="ytpool", bufs=2))
            obpool = ctx.enter_context(tc.tile_pool(name="obpool", bufs=2))
            bsmall = ctx.enter_context(tc.tile_pool(name="bsmall", bufs=2))
            # PSUM (8 banks): projp 2 (qp/kvp) + scpool 2x2 (scores pairs,
            # transposes, C-proj all share the "sc" tag) + yp0/yp1 2
            projp = ctx.enter_context(
                tc.tile_pool(name="projp", bufs=2, space="PSUM"))
            scpool = ctx.enter_context(
                tc.tile_pool(name="scpool", bufs=2, space="PSUM"))
            yppool = ctx.enter_context(
                tc.tile_pool(name="yppool", bufs=1, space="PSUM"))

            def phase_a(sc):
                x_tiles = xpool.tile([128, ND, 512], F32R, tag="x")
                for dt in range(ND):
                    nc.sync.dma_start(
                        x_tiles[:, dt, :], xT[:, dt, ts(sc, 512)]
                    )

                for sl in range(4):
                    st = sc * 4 + sl

                    # --- Q proj: [128 s, 512 qdim] ---
                    qp = projp.tile([128, 512], F32, tag="pj")
                    for dt in range(ND):
                        nc.tensor.matmul(
                            qp[:], x_tiles[:, dt, ts(sl, 128)], wqt_sb[:, dt, :],
                            start=(dt == 0), stop=(dt == ND - 1),
                        )
                    # --- K|V proj: [128 s, 256] ---
                    kvp = projp.tile([128, 512], F32, tag="pj")
                    for dt in range(ND):
                        nc.tensor.matmul(
                            kvp[:, 0:256], x_tiles[:, dt, ts(sl, 128)],
                            wkvt_sb[:, dt, :],
                            start=(dt == 0), stop=(dt == ND - 1),
                        )

                    # --- q+k rmsnorm factors ---
                    q3 = qp.rearrange("p (h d) -> p h d", d=HD)
                    k3 = kvp[:, 0:128].rearrange("p (h d) -> p h d", d=HD)
                    sq = atmp.tile([128, NQH, HD], F32, tag="sq")
                    nc.scalar.square(sq[:], q3)
                    sqk = atmp.tile([128, NKV, HD], F32, tag="sqk")
                    nc.scalar.square(sqk[:], k3)
                    ssum = atmp.tile([128, NQH + NKV], F32, tag="ssum")
                    nc.vector.reduce_sum(
                        ssum[:, 0:NQH].rearrange("p h -> p h ()"),
                        sq[:], axis=mybir.AxisListType.X,
                    )
                    nc.vector.reduce_sum(
                        ssum[:, NQH:].rearrange("p h -> p h ()"),
                        sqk[:], axis=mybir.AxisListType.X,
                    )
                    # rsqrt(ssum + 64*eps) via bit-magic + 2 Newton steps,
                    # all on DVE: keeps the Act engine exclusively on the
                    # Exp/Square table set (single table load for the whole
                    # kernel). rsqrt(mean+eps) = 8*rsqrt(sum+64*eps); the *8
                    # is folded into gain (q) / the kn multiply (k).
                    NW = NQH + NKV
                    aa = atmp.tile([128, NW], F32, tag="aa")
                    nc.vector.tensor_scalar_add(aa[:], ssum[:], HD * RMS_EPS)
                    yy = atmp.tile([128, NW], F32, tag="yy")
                    hshift = atmp.tile([128, NW], mybir.dt.int32, tag="hs")
                    nc.vector.tensor_scalar(
                        hshift[:], aa.bitcast(mybir.dt.int32), 1, None,
                        mybir.AluOpType.arith_shift_right,
                    )
                    nc.vector.tensor_scalar(
                        yy.bitcast(mybir.dt.int32), hshift[:], -1, 0x5F3759DF,
                        mybir.AluOpType.mult, mybir.AluOpType.add,
                    )
                    for _ in range(2):
                        y2 = atmp.tile([128, NW], F32, tag="y2")
                        nc.vector.tensor_mul(y2[:], yy[:], yy[:])
                        nc.vector.tensor_mul(y2[:], y2[:], aa[:])
                        nc.vector.tensor_scalar(
                            y2[:], y2[:], -0.5, 1.5,
                            mybir.AluOpType.mult, mybir.AluOpType.add,
                        )
                        nc.vector.tensor_mul(yy[:], yy[:], y2[:])
                    rr = yy
                    rr2 = atmp.tile([128, NQH], F32, tag="rr2")
                    nc.vector.tensor_mul(rr2[:], rr[:, 0:NQH], gain_sb[:])
                    qn = atmp.tile([128, NQH, HD], F32, tag="qn")
                    nc.vector.tensor_tensor(
                        qn[:], q3, rr2[:, :, None].to_broadcast((128, NQH, HD)),
                        mybir.AluOpType.mult,
                    )
                    # --- q rope (DVE) ---
                    cb = cos_sb[:, st, None, :].to_broadcast((128, NQH, 32))
                    sb = sin_sb[:, st, None, :].to_broadcast((128, NQH, 32))
                    qr = atmp.tile([128, NQH, HD], F32, tag="qr")
                    t1 = atmp.tile([128, NQH, 32], F32, tag="t1")
                    t2 = atmp.tile([128, NQH, 32], F32, tag="t2")
                    nc.vector.tensor_mul(t1[:], qn[:, :, 0:32], cb)
                    nc.vector.tensor_mul(t2[:], qn[:, :, 32:64], sb)
                    nc.vector.tensor_add(qr[:, :, 0:32], t1[:], t2[:])
                    t3 = atmp.tile([128, NQH, 32], F32, tag="t3")
                    t4 = atmp.tile([128, NQH, 32], F32, tag="t4")
                    nc.vector.tensor_mul(t3[:], qn[:, :, 0:32], sb)
                    nc.vector.tensor_mul(t4[:], qn[:, :, 32:64], cb)
                    nc.vector.tensor_tensor(
                        qr[:, :, 32:64], t4[:], t3[:], mybir.AluOpType.subtract
                    )
                    # --- transpose q -> QT (4 into one psum bank, 1 copy) ---
                    qr2 = qr.rearrange("p h d -> p (h d)")
                    tp = scpool.tile([128, 2, 512], F32, tag="sc")
                    for mt in range(NMT):
                        nc.tensor.transpose(
                            tp[:, 0, ts(mt, 128)], qr2[:, ts(mt, 128)], ident[:]
                        )
                    nc.vector.tensor_copy(
                        qt_sb[:, 0:NMT, ts(st, 128)],
                        tp[:, 0, :].rearrange("p (mt s) -> p mt s", s=128),
                    )

                    # --- k rmsnorm apply: kn = (k3 * 8) * rsqrt_sum ---
                    kn = atmp.tile([128, NKV, HD], F32, tag="kn")
                    nc.vector.scalar_tensor_tensor(
                        kn[:], k3, 8.0,
                        rr[:, NQH:, None].to_broadcast((128, NKV, HD)),
                        mybir.AluOpType.mult, mybir.AluOpType.mult,
                    )
                    # --- V (stays [s, d]); read kvp before tpk reuses its
                    # psum slot ---
                    nc.vector.tensor_copy(v_sb[:, st, 0:64], kvp[:, 128:192])
                    nc.vector.tensor_copy(v_sb[:, st, 65:129], kvp[:, 192:256])

                    # --- k rope (Pool) ---
                    cbk = cos_sb[:, st, None, :].to_broadcast((128, NKV, 32))
                    sbk = sin_sb[:, st, None, :].to_broadcast((128, NKV, 32))
                    kr = atmp.tile([128, NKV, HD], F32, tag="kr")
                    u1 = atmp.tile([128, NKV, 32], F32, tag="u1")
                    u2 = atmp.tile([128, NKV, 32], F32, tag="u2")
                    nc.gpsimd.tensor_mul(u1[:], kn[:, :, 0:32], cbk)
                    nc.gpsimd.tensor_mul(u2[:], kn[:, :, 32:64], sbk)
                    nc.gpsimd.tensor_add(kr[:, :, 0:32], u1[:], u2[:])
                    u3 = atmp.tile([128, NKV, 32], F32, tag="u3")
                    u4 = atmp.tile([128, NKV, 32], F32, tag="u4")
                    nc.gpsimd.tensor_mul(u3[:], kn[:, :, 0:32], sbk)
                    nc.gpsimd.tensor_mul(u4[:], kn[:, :, 32:64], cbk)
                    nc.gpsimd.tensor_tensor(
                        kr[:, :, 32:64], u4[:], u3[:], mybir.AluOpType.subtract
                    )
                    # --- transpose k -> KT (bank 2 of the same sc tile) ---
                    kr2 = kr.rearrange("p h d -> p (h d)")
                    nc.tensor.transpose(tp[:, 1, 0:128], kr2, ident[:])
                    nc.vector.tensor_copy(kt_sb[:, ts(st, 128)], tp[:, 1, 0:128])

            yt_tiles = {}

            def phase_b(qt):
                # B: attention for q-tile qt, both kv groups at once
                yp0 = yppool.tile([65, 512], F32, tag="yp0")
                yp1 = yppool.tile([65, 512], F32, tag="yp1")
                qth0 = qt_sb[0:64, 0:NMT, ts(qt, 128)]
                qth1 = qt_sb[64:128, 0:NMT, ts(qt, 128)]
                for jt in range(qt + 1):
                    sp = scpool.tile([128, 2, 512], F32, tag="sc")
                    nc.tensor.matmul(
                        sp[:, 0, :], kt_sb[0:64, ts(jt, 128)], qth0,
                        start=True, stop=True,
                    )
                    nc.tensor.matmul(
                        sp[:, 1, :], kt_sb[64:128, ts(jt, 128)], qth1,
                        start=True, stop=True,
                    )
                    p = ppool.tile([128, 2, 512], F32R, tag="p")
                    nc.scalar.activation(
                        p[:], sp[:], mybir.ActivationFunctionType.Exp
                    )
                    if jt == qt:
                        p4 = p.rearrange("p g (m s) -> p g m s", s=128)
                        nc.gpsimd.tensor_tensor(
                            p4,
                            p4,
                            mask_sb[:, None, None, :]
                            .to_broadcast((128, 2, NMT, 128)),
                            mybir.AluOpType.mult,
                        )
                    nc.tensor.matmul(
                        yp0[:], v_sb[:, jt, 0:65], p[:, 0, :],
                        start=(jt == 0), stop=(jt == qt),
                    )
                    nc.tensor.matmul(
                        yp1[:], v_sb[:, jt, 65:130], p[:, 1, :],
                        start=(jt == 0), stop=(jt == qt),
                    )
                # normalize: y / denom (denom = row 64)
                yt = ytpool.tile([128, NMT, 128], F32R, tag="yt")
                for g, yp in ((0, yp0), (1, yp1)):
                    rrow = bsmall.tile([1, 512], F32, tag=f"rrow{g}")
                    nc.vector.reciprocal(rrow[:], yp[64:65, :])
                    bs = bsmall.tile([64, 512], F32, tag=f"bs{g}")
                    nc.gpsimd.partition_broadcast(bs[:], rrow[:])
                    nc.vector.tensor_tensor(
                        yt[ts(g, 64), 0:NMT, :]
                        .rearrange("p m s -> p (m s)"),
                        yp[0:64, :], bs[:], mybir.AluOpType.mult,
                    )
                yt_tiles[qt] = yt

            def phase_c(qt):
                yt = yt_tiles.pop(qt)
                op = scpool.tile([128, 2, 512], F32, tag="sc")
                for mt in range(NMT):
                    nc.tensor.matmul(
                        op[:, 0, :], yt[:, mt, :], wpt_sb[:, mt, 0:512],
                        start=(mt == 0), stop=(mt == NMT - 1),
                    )
                for mt in range(NMT):
                    nc.tensor.matmul(
                        op[:, 1, :], yt[:, mt, :], wpt_sb[:, mt, 512:1024],
                        start=(mt == 0), stop=(mt == NMT - 1),
                    )
                ob = obpool.tile([128, DIM], F32, tag="ob")
                nc.vector.tensor_copy(ob[:], op.rearrange("p b f -> p (b f)"))
                nc.sync.dma_start(out[ts(qt, 128), :], ob[:])

            # C is emitted one q-tile behind B so its yt dependency is
            # already satisfied when the in-order PE queue reaches it.
            def body():
                setup()
                for sc in range(NSC):
                    phase_a(sc)
                    for sl in range(4):
                        qt = sc * 4 + sl
                        phase_b(qt)
                        if qt > 0:
                            phase_c(qt - 1)
                phase_c(NST - 1)

            # `reps` complete forward passes per dispatch, unrolled (used by
            # the bench to amortize the per-dispatch host/tunnel overhead out
            # of the per-iteration measurement; unrolling also lets adjacent
            # passes overlap across engines, unlike a barriered hw loop)
            for _ in range(reps):
                body()

    nc.compile()
    return nc


def _rope_tables():
    inv = (
        1.0 / (np.float32(10000.0) ** (np.arange(0, HD, 2, dtype=np.float32) / np.float32(HD)))
    ).astype(np.float32)
    freqs = np.arange(S, dtype=np.float32)[:, None] * inv[None, :]
    return np.cos(freqs).astype(np.float32), np.sin(freqs).astype(np.float32)


def _masks():
    k = np.arange(128)[:, None]
    q = np.arange(128)[None, :]
    return (k <= q).astype(np.float32)


HEAD_PERM = [0, 4, 1, 5, 2, 6, 3, 7]


def round_f32r(a):
    """Round fp32 to FP32R (11-bit mantissa), round-to-nearest-even."""
    u = np.ascontiguousarray(a, np.float32).view(np.uint32)
    u = u + 0x7FF + ((u >> 12) & 1)
    u &= np.uint32(0xFFFFF000)
    return u.view(np.float32)


def in_map_for_core(c, x, Wq, Wk, Wv, Wproj, q_gain, cos, sin, masks):
    b, hh = c // 2, c % 2
    g_sh = q_gain[8 * hh : 8 * hh + 8][HEAD_PERM]
    g8 = np.repeat(g_sh[None, :], 128, axis=0)
    wq_sh = (
        Wq[512 * hh : 512 * hh + 512, :].reshape(8, 64, DIM)[HEAD_PERM]
    ).reshape(512, DIM)
    parts = {
        "xT": round_f32r(x[b].T),
        "wqt": round_f32r(wq_sh.T),
        "wkvt": round_f32r(
            np.concatenate(
                [
                    Wk[128 * hh : 128 * hh + 128, :],
                    Wv[128 * hh : 128 * hh + 128, :],
                ],
                axis=0,
            ).T
        ),
        "wpt": round_f32r(
            Wproj[:, 512 * hh : 512 * hh + 512]
            .T.reshape(8, 64, DIM)[HEAD_PERM]
            .reshape(512, DIM)
        ),
        "gain8": np.ascontiguousarray(g8.astype(np.float32)),
        "cosd": cos,
        "sind": sin,
        "maskd": masks,
    }
    blob = np.empty(BLOB_TOTAL, np.float32)
    for n, _ in _BLOB_SIZES:
        a, sz = BLOB_OFFS[n]
        blob[a : a + sz] = np.ascontiguousarray(parts[n], np.float32).ravel()
    return {"blob": blob}


def kernel(x, Wq, Wk, Wv, Wproj, q_gain):
    x = np.asarray(x, np.float32)
    Wq = np.asarray(Wq, np.float32)
    Wk = np.asarray(Wk, np.float32)
    Wv = np.asarray(Wv, np.float32)
    Wproj = np.asarray(Wproj, np.float32)
    q_gain = np.asarray(q_gain, np.float32)

    if "runner" not in _CACHE:
        _CACHE["runner"] = _Runner(build_program())
    runner = _CACHE["runner"]

    cos, sin = _rope_tables()
    masks = _masks()

    in_maps = [
        in_map_for_core(c, x, Wq, Wk, Wv, Wproj, q_gain, cos, sin, masks)
        for c in range(8)
    ]

    # The axon tunnel occasionally corrupts a run (stale/dropped buffers on
    # the first execution after compile). The kernel is deterministic, so
    # run twice and accept only when both executions agree bit-exactly;
    # retry otherwise.
    prev = None
    for _attempt in range(4):
        results = runner.run(in_maps)
        cur = np.stack([results[c]["out"] for c in range(8)])
        if prev is not None and np.array_equal(prev, cur):
            break
        prev = cur

    out = np.empty((4, S, DIM), np.float32)
    for b in range(4):
        out[b] = prev[2 * b] + prev[2 * b + 1]
    return out


class _Runner:
    """Cached fast-dispatch jit of the SPMD bass program on 8 axon cores."""

    def __init__(self, nc, n_cores=8):
        import jax
        from jax.experimental.shard_map import shard_map
        from jax.sharding import Mesh, NamedSharding, PartitionSpec

        from concourse import bass2jax

        bass2jax.install_neuronx_cc_hook()
        self.nc = nc
        self.n_cores = n_cores
        in_names: list[str] = []
        out_names: list[str] = []
        out_avals = []
        zero_outs = []
        part_name = nc.partition_id_tensor.name if nc.partition_id_tensor else None
        for alloc in nc.m.functions[0].allocations:
            if not isinstance(alloc, mybir.MemoryLocationSet):
                continue
            name = alloc.memorylocations[0].name
            if alloc.kind == "ExternalInput":
                if name != part_name:
                    in_names.append(name)
            elif alloc.kind == "ExternalOutput":
                out_names.append(name)
                shape = tuple(alloc.tensor_shape)
                dtype = mybir.dt.np(alloc.dtype)
                out_avals.append(jax.core.ShapedArray(shape, dtype))
                zero_outs.append(np.zeros(shape, dtype))
        all_names = list(in_names) + list(out_names)
        if part_name is not None:
            all_names.append(part_name)
        self.in_names = in_names
        self.out_names = out_names
        self.out_avals = out_avals
        self.zero_outs = zero_outs

        def _body(*args):
            operands = list(args)
            if part_name is not None:
                operands.append(bass2jax.partition_id_tensor())
            outs = bass2jax._bass_exec_p.bind(
                *operands,
                out_avals=tuple(out_avals),
                in_names=tuple(all_names),
                out_names=tuple(out_names),
                lowering_input_output_aliases=(),
                sim_require_finite=True,
                sim_require_nnan=True,
                nc=nc,
            )
            return tuple(outs)

        devices = jax.devices()[:n_cores]
        self.mesh = Mesh(np.asarray(devices), ("core",))
        self.sharding = NamedSharding(self.mesh, PartitionSpec("core"))
        n_params = len(in_names) + len(out_avals)
        self._fn = shard_map(
            _body,
            mesh=self.mesh,
            in_specs=(PartitionSpec("core"),) * n_params,
            out_specs=(PartitionSpec("core"),) * len(out_avals),
            check_rep=False,
        )
        self._bass2jax = bass2jax
        self._jax = jax
        self.compiled = None

    def _concat_inputs(self, in_maps):
        return [
            np.concatenate(
                [np.asarray(in_maps[c][n]) for c in range(self.n_cores)], axis=0
            )
            for n in self.in_names
        ]

    def _concat_zeros(self):
        return [
            np.zeros((self.n_cores * z.shape[0], *z.shape[1:]), z.dtype)
            for z in self.zero_outs
        ]

    def _get_compiled(self, ins_dev, zeros_dev):
        if self.compiled is None:
            jax = self._jax
            self.compiled = self._bass2jax.fast_dispatch_compile(
                lambda: jax.jit(self._fn, keep_unused=True)
                .lower(*ins_dev, *zeros_dev)
                .compile()
            )
        return self.compiled

    def run(self, in_maps):
        jax = self._jax
        ins_dev = [jax.device_put(a, self.sharding) for a in self._concat_inputs(in_maps)]
        zeros_dev = [jax.device_put(z, self.sharding) for z in self._concat_zeros()]
        fn = self._get_compiled(ins_dev, zeros_dev)
        out_arrs = fn(*ins_dev, *zeros_dev)
        return [
            {
                n: np.asarray(out_arrs[i]).reshape(
                    self.n_cores, *self.out_avals[i].shape
                )[c]
                for i, n in enumerate(self.out_names)
            }
            for c in range(self.n_cores)
        ]

    def bench(self, in_maps, iters=500, reps=3):
        """Average wall time per execution with device-resident inputs,
        deep-pipelined to amortize tunnel latency. Output refs are dropped
        as calls are dispatched (executions are in-order per device, so
        blocking on the final output times the whole pipeline) to bound
        device memory. Returns the fastest of `reps` repetitions."""
        import time

        jax = self._jax
        ins_dev = [jax.device_put(a, self.sharding) for a in self._concat_inputs(in_maps)]
        zeros_dev = [jax.device_put(z, self.sharding) for z in self._concat_zeros()]
        fn = self._get_compiled(ins_dev, zeros_dev)
        r = fn(*ins_dev, *zeros_dev)
        jax.block_until_ready(r)
        r = None
        best = None
        for _ in range(reps):
            t0 = time.time()
            out = None
            for _ in range(iters):
                out = fn(*ins_dev, *zeros_dev)
            jax.block_until_ready(out)
            dt = (time.time() - t0) / iters
            out = None
            if best is None or dt < best:
                best = dt
        return best


def _build_runner(nc):
    return _Runner(nc)

